# revision 7
# baseline (speedup 1.0000x reference)
"""Contour -> distance map kernel for 8 Trainium2 NeuronCores.

Math (per polygon p, pixel m, edge k, with vertex v_k and next vertex v_{k+1}):
  diff_k = v_k - m,  roll_k = v_{k+1} - m
  n2_k    = |diff_k|^2
  dot_k   = diff_k . roll_k
  cross_k = diff_k x roll_k
All three are affine in phi(m) = [1, mx, my, mx^2+my^2], so one K=4 matmul
per 128-pixel tile produces (n2 | dot | cross) for all 64 edges.

Reference angle chain  arccos(clip(dot/(nd*nr), -1+eps, 1-eps))  is rewritten
division-light:  theta_k = pi/2 - arctan(clamp(dot/cross, +/-C)),
C = cot(arccos(1-eps)), and
  sum_k tanh(1e5*cross)*theta = (pi/2)*sum(sgn) - sum(|sgn|*arctan(tc))
exactly (signs fold through arctan's oddness).  The DVE min/max clamp
suppresses NaN/Inf from cross==0 reciprocals; those terms are killed by
|sgn|~0 anyway.

Device outputs per core (1 polygon each): SS = sum(sgn), SA = sum(|sgn|*at),
MN = min(n2) as (128, 512) arrays [partition = pixel%128, col = pixel//128].
Host epilogue: wind = |pi/2*SS - SA|/(2pi), prod = wind*sqrt(MN),
dmap = prod / global_max(prod).
"""

import numpy as np

SIZE = 256
EPS = 1e-5
NCORES = 8
K = 64
NTILE = 512          # 128-pixel tiles per core
BATCH = 8            # tiles per batch
NBATCH = NTILE // BATCH
CLAMP = float((1.0 - EPS) / np.sqrt(1.0 - (1.0 - EPS) ** 2))  # 223.607...

_CACHE = {}
LAST_RESULTS = None


def _register_custom_ops():
    """Two fused DVE ops (registered into concourse.dve_ops at runtime):

    MULT_CLAMP_ANT: out = max(min(in0*in1, C0), C1)      [3 ALU stages]
        -> tc = clamp(dot * (1/cross), +/-CLAMP).  HW min/max suppress NaN,
        sanitizing 1/0 = NaN/Inf garbage from the reciprocal.
    SGN_THETA_ANT:  out = in0*C0 - |in0|*in1             [5 stages]
        -> c = sgn*(pi/2) - |sgn|*arctan(tc), |x| = max(x, 0-x) on v3.
    """
    import numpy as np
    import concourse.dve_ops as dve_ops
    from concourse.dve_ops import DveOp
    from concourse.dve_spec import Spec, Src0, Src1, C0, C1, Zero, maxx, minn, lower
    from concourse.dve_uop import DveOpSpec

    if "MULT_CLAMP_ANT" in dve_ops._SUB_OPCODE_FOR_NAME:
        return

    def _make(name, spec):
        row = max(dve_ops._SUB_OPCODE_FOR_NAME.values()) + 1
        assert row < 0x20
        dve_ops._SUB_OPCODE_FOR_NAME[name] = row
        shas = {}
        for ver in ("v3", "v4"):
            try:
                uops = lower(spec, ver=ver)
                shas[ver] = DveOpSpec(name=name, opcode=row, uops=uops,
                                      rd1_en=True).sha(ver)
            except Exception:
                pass
        op = DveOp(name, spec, subdim=False, uops_sha=shas)
        dve_ops.OPS.append(op)
        dve_ops.CUSTOM_DVE_SPECS[name] = spec
        return op

    mc = _make(
        "MULT_CLAMP_ANT",
        Spec(
            body=maxx(minn(Src0 * Src1, C0), C1),
            reference=lambda in0, in1, c0, c1, c2: np.maximum(
                np.minimum(in0 * in1, c0), c1),
        ),
    )
    st = _make(
        "SGN_THETA_ANT",
        Spec(
            body=Src0 * C0 - maxx(Src0, Zero - Src0) * Src1,
            reference=lambda in0, in1, c0, c1, c2: in0 * c0
            - np.abs(in0) * in1,
        ),
    )
    _CACHE["ops"] = (mc, st)


def _build_program():
    import concourse.bacc as bacc
    import concourse.tile as tile
    from concourse import mybir
    import concourse.bass as bass

    _register_custom_ops()
    mc_op, st_op = _CACHE["ops"]

    f32 = mybir.dt.float32
    AF = mybir.ActivationFunctionType
    ALU = mybir.AluOpType

    nc = bacc.Bacc("TRN2", target_bir_lowering=False, debug=False,
                   num_devices=NCORES)

    phiT = nc.dram_tensor("phiT", [4, NTILE, 128], f32, kind="ExternalInput")
    wmat = nc.dram_tensor("wmat", [4, 3 * K], f32, kind="ExternalInput")
    sc_d = nc.dram_tensor("sc", [128, NTILE], f32, kind="ExternalOutput")
    mn_d = nc.dram_tensor("mn", [128, NTILE], f32, kind="ExternalOutput")

    CHUNK = 64   # tiles of phiT per staged DMA chunk
    SUPER = 4    # batches per super-batch (SBUF-side op granularity)
    SB_T = SUPER * BATCH  # tiles per super-batch (32)

    with tile.TileContext(nc) as tc:
        with (
            tc.tile_pool(name="wpool", bufs=1) as wpool,
            tc.tile_pool(name="chunkpool", bufs=2) as chunkpool,
            tc.tile_pool(name="psum", bufs=2, space="PSUM") as psum_pool,
            tc.tile_pool(name="work", bufs=3) as work,
            tc.tile_pool(name="wide", bufs=2) as wide,
            tc.tile_pool(name="outs", bufs=1) as outs,
        ):
            w_s = wpool.tile([4, 3 * K], f32)
            nc.sync.dma_start(w_s[:], wmat[:])

            sc_t = outs.tile([128, NTILE], f32)
            mn_t = outs.tile([128, NTILE], f32)

            chunk = None
            for sb in range(NBATCH // SUPER):
                sg_w = wide.tile([128, SB_T, K], f32, tag="sg")
                tc_w = wide.tile([128, SB_T, K], f32, tag="tc")
                for j in range(SUPER):
                    b = sb * SUPER + j
                    if b % (CHUNK // BATCH) == 0:
                        c = b // (CHUNK // BATCH)
                        chunk = chunkpool.tile([4, CHUNK, 128], f32,
                                               tag="chunk")
                        nc.sync.dma_start(
                            chunk[:], phiT[:, c * CHUNK:(c + 1) * CHUNK, :])
                    j0 = (b % (CHUNK // BATCH)) * BATCH

                    pt = psum_pool.tile([128, BATCH, 256], f32, tag="pt")
                    for t in range(BATCH):
                        nc.tensor.matmul(
                            pt[:, t, 0:3 * K],
                            chunk[:, j0 + t, :],
                            w_s[:],
                            start=True, stop=True,
                        )
                    n2 = pt[:, :, 0:K]
                    dot = pt[:, :, K:2 * K]
                    cross = pt[:, :, 2 * K:3 * K]
                    jsl = slice(j * BATCH, (j + 1) * BATCH)

                    nc.scalar.activation(sg_w[:, jsl, :], cross,
                                         AF.Tanh, scale=100000.0)
                    rc = work.tile([128, BATCH, K], f32, tag="rc")
                    nc.vector.reciprocal_approx_fast(rc[:], cross)
                    nc.vector._custom_dve(
                        mc_op, out=tc_w[:, jsl, :], in0=dot, in1=rc[:],
                        s0=CLAMP, s1=-CLAMP)
                    nc.vector.tensor_reduce(
                        mn_t[:, bass.ts(b, BATCH)], n2,
                        axis=mybir.AxisListType.X, op=ALU.min)

                at_w = wide.tile([128, SB_T, K], f32, tag="at")
                nc.scalar.activation(at_w[:], tc_w[:], AF.Arctan)
                c_w = wide.tile([128, SB_T, K], f32, tag="c")
                nc.vector._custom_dve(
                    st_op, out=c_w[:], in0=sg_w[:], in1=at_w[:],
                    s0=float(np.pi / 2), s1=0.0)
                nc.vector.tensor_reduce(
                    sc_t[:, bass.ts(sb, SB_T)], c_w[:],
                    axis=mybir.AxisListType.X, op=ALU.add)

            nc.sync.dma_start(sc_d[:], sc_t[:])
            nc.sync.dma_start(mn_d[:], mn_t[:])

    nc.compile()
    return nc


def _host_inputs(contour):
    """Per-core input maps: shared phi(mesh) lhsT and per-polygon W."""
    C = contour.reshape(NCORES, K, 2).astype(np.float64)
    cx, cy = C[..., 0], C[..., 1]
    cxn, cyn = np.roll(cx, -1, 1), np.roll(cy, -1, 1)
    ones = np.ones_like(cx)
    Wn2 = np.stack([cx * cx + cy * cy, -2 * cx, -2 * cy, ones], 1)
    Wdot = np.stack([cx * cxn + cy * cyn, -(cx + cxn), -(cy + cyn), ones], 1)
    Wcr = np.stack([cy * cxn - cx * cyn, cyn - cy, cx - cxn,
                    np.zeros_like(cx)], 1)
    W = np.concatenate([Wn2, Wdot, Wcr], axis=2).astype(np.float32)  # (8,4,192)

    ax = np.arange(SIZE) / SIZE
    gx, gy = np.meshgrid(ax, ax, indexing="ij")
    mx, my = gx.ravel(), gy.ravel()
    phi = np.stack([np.ones(SIZE * SIZE), mx, my, mx * mx + my * my], 0)
    phiT = phi.astype(np.float32).reshape(4, NTILE, 128)

    return [{"phiT": phiT, "wmat": np.ascontiguousarray(W[p])}
            for p in range(NCORES)]


def _get_executor():
    """Build (once) a reusable jitted SPMD executor over the 8 cores.

    Mirrors concourse.bass2jax.run_bass_via_pjrt but without output-buffer
    donation: every output element is fully written by the kernel, so the
    zero output operands can be uploaded once and reused across calls.
    """
    if "exec" in _CACHE:
        return _CACHE["exec"]

    import jax
    import jax.numpy as jnp  # noqa: F401
    from jax.sharding import Mesh, PartitionSpec, NamedSharding
    from jax.experimental.shard_map import shard_map
    import concourse.mybir as mybir
    from concourse.bass2jax import _bass_exec_p, install_neuronx_cc_hook

    install_neuronx_cc_hook()
    nc = _CACHE["nc"]
    partition_name = (nc.partition_id_tensor.name
                      if nc.partition_id_tensor else None)

    in_names, out_names, out_avals, zero_outs = [], [], [], []
    for alloc in nc.m.functions[0].allocations:
        if not isinstance(alloc, mybir.MemoryLocationSet):
            continue
        name = alloc.memorylocations[0].name
        if alloc.kind == "ExternalInput":
            if name == partition_name:
                continue
            in_names.append(name)
        elif alloc.kind == "ExternalOutput":
            out_names.append(name)
            shape = tuple(alloc.tensor_shape)
            dtype = mybir.dt.np(alloc.dtype)
            out_avals.append(jax.core.ShapedArray(shape, dtype))
            zero_outs.append(np.zeros(shape, dtype))
    n_params = len(in_names)
    all_names = in_names + out_names
    if partition_name is not None:
        all_names = all_names + [partition_name]

    from concourse.bass2jax import partition_id_tensor

    def _body(*args):
        operands = list(args)
        if partition_name is not None:
            operands.append(partition_id_tensor())
        outs = _bass_exec_p.bind(
            *operands,
            out_avals=tuple(out_avals),
            in_names=tuple(all_names),
            out_names=tuple(out_names),
            lowering_input_output_aliases=(),
            sim_require_finite=True,
            sim_require_nnan=True,
            nc=nc,
        )
        return tuple(outs)

    devices = jax.devices()[:NCORES]
    mesh = Mesh(np.asarray(devices), ("core",))
    nspec = (PartitionSpec("core"),) * (n_params + len(out_names))
    sharded = jax.jit(
        shard_map(_body, mesh=mesh, in_specs=nspec,
                  out_specs=(PartitionSpec("core"),) * len(out_names),
                  check_rep=False),
        keep_unused=True,
    )
    sharding = NamedSharding(mesh, PartitionSpec("core"))
    zeros_dev = [
        jax.device_put(
            np.zeros((NCORES * z.shape[0], *z.shape[1:]), z.dtype), sharding)
        for z in zero_outs
    ]
    _CACHE["exec"] = (sharded, sharding, in_names, out_names, zeros_dev)
    return _CACHE["exec"]


def _run(contour):
    """Returns list (per core) of dicts {sa, ss, mn} as np arrays."""
    import jax
    sharded, sharding, in_names, out_names, zeros_dev = _get_executor()
    in_maps = _host_inputs(contour)
    concat = {
        name: np.concatenate([m[name] for m in in_maps], axis=0)
        for name in in_names
    }
    if "phiT_dev" not in _CACHE:
        _CACHE["phiT_dev"] = jax.device_put(concat["phiT"], sharding)
    ins = [
        _CACHE["phiT_dev"] if name == "phiT"
        else jax.device_put(concat[name], sharding)
        for name in in_names
    ]
    outs = sharded(*ins, *zeros_dev)
    res = []
    per_core_rows = {n: concat[n].shape[0] // NCORES for n in in_names}
    del per_core_rows
    for c in range(NCORES):
        d = {}
        for i, name in enumerate(out_names):
            arr = np.asarray(outs[i])
            rows = arr.shape[0] // NCORES
            d[name] = arr[c * rows:(c + 1) * rows]
        res.append(d)
    return res


def benchmark(contour, iters=20):
    """Pipelined repeated execution; returns avg seconds/iteration."""
    import time
    import jax
    sharded, sharding, in_names, out_names, zeros_dev = _get_executor()
    in_maps = _host_inputs(np.asarray(contour, dtype=np.float32))
    concat = {
        name: np.concatenate([m[name] for m in in_maps], axis=0)
        for name in in_names
    }
    ins = [jax.device_put(concat[name], sharding) for name in in_names]
    out = sharded(*ins, *zeros_dev)  # warm-up
    jax.block_until_ready(out)
    t0 = time.time()
    outs = [sharded(*ins, *zeros_dev) for _ in range(iters)]
    jax.block_until_ready(outs[-1])
    t1 = time.time()
    return (t1 - t0) / iters


def kernel(contour, *, _trace=False):
    contour = np.asarray(contour, dtype=np.float32)
    if "nc" not in _CACHE:
        _CACHE["nc"] = _build_program()

    results = _run(contour)

    planes = []
    for p in range(NCORES):
        out = results[p]
        S = out["sc"].T.ravel()
        mn = out["mn"].T.ravel()
        wind = np.abs(S * np.float32(1.0 / (2 * np.pi)))
        dist = np.sqrt(np.maximum(mn, np.float32(0)))
        planes.append((wind * dist).astype(np.float32))
    prod = np.stack(planes)                      # (8, 65536)
    dmap = (prod / prod.max()).astype(np.float32)
    return dmap.reshape(2, 4, SIZE, SIZE)
